# revision 56
# baseline (speedup 1.0000x reference)
"""AllAttention kernel for Trainium2 (8 NeuronCores, pure data parallel).

Computation (per batch item b):
    att   = feats[b] @ Wf + bf            # [A, H]
    att_h = h[b] @ Wh + bh                # [H]
    dot   = tanh(att + att_h)             # [A, H]
    s     = dot @ wa (+ ba)               # [A]   (ba dropped: softmax shift-invariant)
    w     = softmax(s)                    # [A]
    out   = w @ feats[b]                  # [R]

Shapes: B=256, A=196, R=1024, H=512. Sharded: batch/8 per core (32 each).

Final design (pairs of batch items flow through a software pipeline):
  - Replicated weights host-packed: Wf quantized to fp8 e4m3 at x32 in a
    DoubleRow shuffle ([p,hc,c,kt,m] = 32*Wf[256c+2p+kt, 128hc+m]); Wh/wa
    bf16 in hc-major SBUF layout; bf+bh pre-summed.
  - feats stream in once as casting SWDGE DMAs (fp32->bf16, the cost of
    the load is the bf16 write), then DVE casts bf16->fp8. fb0 = a[0:128],
    fb1 = a[100:196] (96 rows: 16-aligned for the xbar AND 32-aligned for
    engine partition bases; overlaps fb0 by 28 rows so no pad/memset).
  - The fp8 tiles, viewed as packed u16 pairs (adjacent r values in one
    element), go through the DMA xbar transpose at HALF the bf16 tile
    count; the byte pairing lands exactly in DoubleRow's k-tile layout.
    Outputs are fully contiguous 512B-aligned tiles - partial-segment DMA
    writes read-modify-write 512B segments, so concurrent strided writes
    into shared tiles corrupt each other (this bit us).
  - mm1 att^T = Wf^T feats^T in fp8 DoubleRow: 16 matmuls per (pair, hc),
    each contracting 256 r values; one PSUM-arming start per bank. tanh
    fused on ACT with per-partition beta bias and the 1/32 fold-back
    (beta = h@Wh + bf + bh computed on PE as the Wh chunks land).
  - scores via PE (wa columns stationary), exp on ACT with accum_out into
    per-group sums. Softmax normalization is deferred: mm2 runs on raw
    exp weights; the drain rescales by 1/sum (reciprocals DVE'd and
    scattered to partitions {0,32,64,96} via a tiny ACT-queue DMA).
  - exp row -> PE transposes -> one ACT copy -> ec bf16 columns; mm2 with
    M=1 weights into 4 persistent pre-zeroed PSUM banks at partitions
    {0,32,64,96} via tile_position, split a=0:100 / 100:196 (partition
    base 0 for every operand).
  - The tail of pair q interleaves into pair q+2's mm1 (LAG=2): after the
    fp8 speedup a single pair of mm1 no longer hides the ACT exp/ec
    latency chain.
  - Engine/queue separation: SWDGE = feats loads; SP hwdge = transposes;
    ACT hwdge = reciprocal scatter + output stores (so stores never
    head-block transposes); DVE = fp8 casts + reciprocals only; drains
    are scheduler-deprioritized so they never sandwich between the
    latency-critical tanh/exp pairs on ACT.
  - PE warmup matmuls cover the startup DMA prefill and early pipeline
    bubbles: any PE idle gap drops the tensor engine p-state to 1/3.7
    speed for the next 3us (calibrated cost model), so idle is doubly
    expensive.
"""

import os
from contextlib import ExitStack

import numpy as np
import ml_dtypes

import concourse.bass as bass
import concourse.bacc as bacc
import concourse.mybir as mybir
import concourse.tile as tile
from concourse.bass import ds, ts
from concourse.bass_utils import run_bass_kernel_spmd
from concourse.masks import make_identity

F32 = mybir.dt.float32
BF16 = mybir.dt.bfloat16
FP8 = mybir.dt.float8e4
U16 = mybir.dt.uint16
DROW = mybir.MatmulPerfMode.DoubleRow
TANH = mybir.ActivationFunctionType.Tanh
EXP = mybir.ActivationFunctionType.Exp
COPY = mybir.ActivationFunctionType.Copy

WF_SCALE = 32.0            # Wf is quantized to fp8 at x32; tanh scale undoes it

B, A, R, H = 256, 196, 1024, 512
N_CORES = 8
BL = B // N_CORES          # 32 batch items per core
A0 = 128                   # a-rows 0:128
A1 = 96                    # a-rows 100:196 (overlaps A0 by 28)
A1_OFF = A - A1            # 100
RC = R // 128              # 8 r-chunks
HC = H // 128              # 4 h-chunks
NPAIRS = BL // 2

N_WARM0 = 8                # PE warm matmuls before h transposes
N_WARM1 = 22               # PE warm matmuls covering the DMA prefill
# warm matmuls after early pairs: fill the startup DMA-backlog bubbles so
# the PE p-state never drops while the feats pipeline catches up
WARM_AFTER = {0: 30, 1: 18, 2: 12, 3: 8, 4: 6, 5: 4, 6: 4, 7: 4, 8: 4, 9: 4}

# tuning knobs (env-overridable for sweeps; defaults = shipped config)
_e = os.environ.get
FB_BUFS = int(_e("KERN_FB_BUFS", "6"))
F8_BUFS = int(_e("KERN_F8_BUFS", "3"))
FTP_BUFS = int(_e("KERN_FTP_BUFS", "3"))
L_HOOK = int(_e("KERN_L_HOOK", "1"))     # mm1-chunk index where loads are emitted
T_HOOK = int(_e("KERN_T_HOOK", "2"))     # mm1-chunk index where transposes are emitted
L_AHEAD = int(_e("KERN_L_AHEAD", "3"))   # pairs of load lookahead
T_AHEAD = int(_e("KERN_T_AHEAD", "2"))   # pairs of transpose lookahead
LAG = int(_e("KERN_LAG", "2"))           # pairs between mm1 and its tail
PERIOD_NS = int(_e("KERN_PERIOD", "0"))  # manual schedule floor per pair (0=off)
T0_NS = int(_e("KERN_T0", "9000"))       # floor offset for pair 0
if _e("KERN_WARMS"):
    WARM_AFTER = {i: int(v) for i, v in enumerate(_e("KERN_WARMS").split(","))}

rows4 = slice(0, 97, 32)   # partitions {0, 32, 64, 96} (DMA gather only)


def _emit(tc):
    nc = tc.nc
    ctx = ExitStack()

    h_d = nc.dram_tensor("h_in", [BL, R], F32, kind="ExternalInput").ap()
    feats_d = nc.dram_tensor("feats_in", [BL, A, R], F32, kind="ExternalInput").ap()
    wf_d = nc.dram_tensor("wf_in", [128, HC, 4, 2, 128], FP8, kind="ExternalInput").ap()
    wh_d = nc.dram_tensor("wh_in", [128, HC, RC, 128], BF16, kind="ExternalInput").ap()
    bfh_d = nc.dram_tensor("bfh_in", [1, H], BF16, kind="ExternalInput").ap()
    wa_d = nc.dram_tensor("wa_in", [128, HC], BF16, kind="ExternalInput").ap()
    out_d = nc.dram_tensor("out", [BL, R], F32, kind="ExternalOutput").ap()

    singles = ctx.enter_context(tc.tile_pool(name="singles", bufs=1))

    ident = singles.tile([128, 128], F32)
    make_identity(nc, ident)

    wa_sb = singles.tile([128, HC], BF16)        # wa[128*c + p] -> [p, c]
    betaT = singles.tile([128, HC, BL], F32)     # beta^T[h, b] per h-chunk
    # Wf fp8 (x32), shuffled for DoubleRow: [p, hc, c, kt, m] = Wf[256c+2p+kt, 128hc+m]
    wf8 = singles.tile([128, HC, 4, 2, 128], FP8)
    wh_bf = singles.tile([128, HC, RC, 128], BF16)
    bfh = singles.tile([1, H], BF16)
    hT = singles.tile([128, RC, BL], BF16)
    ones_row = singles.tile([1, BL], BF16)
    dmy = singles.tile([128, 256], BF16)         # warmup rhs
    # full 512B rows so the tiny scatter DMA's read-modify-write segment
    # never spans a neighboring, concurrently-written tile
    rcol = [singles.tile([128, 128], F32, tag=f"rcol{i}", name=f"rcol{i}") for i in range(2)]

    # ---- pools ----
    fb0p = ctx.enter_context(tc.tile_pool(name="fb0p", bufs=FB_BUFS))
    fb1p = ctx.enter_context(tc.tile_pool(name="fb1p", bufs=FB_BUFS))
    f8p0 = ctx.enter_context(tc.tile_pool(name="f8p0", bufs=F8_BUFS))
    f8p1 = ctx.enter_context(tc.tile_pool(name="f8p1", bufs=F8_BUFS))
    ftp = ctx.enter_context(tc.tile_pool(name="ftp", bufs=FTP_BUFS))
    dtp = ctx.enter_context(tc.tile_pool(name="dtp", bufs=3))
    erow = ctx.enter_context(tc.tile_pool(name="erow", bufs=2))
    ecp = ctx.enter_context(tc.tile_pool(name="ecp", bufs=2))
    gsp = ctx.enter_context(tc.tile_pool(name="gsp", bufs=2))
    stp = ctx.enter_context(tc.tile_pool(name="stp", bufs=4))

    mp_ps = ctx.enter_context(tc.tile_pool(name="mp_ps", bufs=2, space="PSUM"))
    sc_ps = ctx.enter_context(tc.tile_pool(name="sc_ps", bufs=2, space="PSUM"))
    res_ps = ctx.enter_context(tc.tile_pool(name="res_ps", bufs=1, space="PSUM"))

    res_tiles = [res_ps.tile([128, 512], F32, tag=f"res{i}", name=f"res{i}") for i in range(4)]

    # ---------------- DMA emission helpers ----------------
    def load_pair(p):
        """Casting loads (fp32 -> bf16) for pair p on the SWDGE queue, plus
        DVE bf16 -> fp8 casts feeding the mm1 transpose path."""
        b0 = 2 * p
        f0 = fb0p.tile([A0, 2, R], BF16, tag="fb0")
        f1 = fb1p.tile([A1, 2, R], BF16, tag="fb1")
        nc.gpsimd.dma_start(
            out=f0, in_=feats_d[b0 : b0 + 2, 0:A0, :].rearrange("s p r -> p s r")
        )
        nc.gpsimd.dma_start(
            out=f1, in_=feats_d[b0 : b0 + 2, A1_OFF:A, :].rearrange("s p r -> p s r")
        )
        f08 = f8p0.tile([A0, 2, R], FP8, tag="f08")
        f18 = f8p1.tile([A1, 2, R], FP8, tag="f18")
        nc.vector.tensor_copy(out=f08, in_=f0)
        nc.vector.tensor_copy(out=f18, in_=f1)
        return f0, f1, f08, f18

    def xpose_pair(fb):
        """Two xbar transposes of the fp8 data viewed as packed u16 pairs
        (adjacent r values travel together), into fully contiguous,
        512B-aligned tiles. tp8[p, t, j, kt] = fp8 feats[a=j, r=256*(t%4)
        + 2p + kt] for t = s*4 + c; tq8 likewise with a = 100+j.
        The (j, kt) byte layout is exactly DoubleRow's k-tile pairing."""
        f0, f1, f08, f18 = fb
        tp8 = ftp.tile([128, 2 * HC, A0], U16, tag="tp")
        tq8 = ftp.tile([128, 2 * HC, A1], U16, tag="tq")
        nc.sync.dma_start(
            out=tp8,
            in_=f08.rearrange("p s r -> p (s r)").bitcast(U16),
            transpose=True,
        )
        nc.sync.dma_start(
            out=tq8,
            in_=f18.rearrange("p s r -> p (s r)").bitcast(U16),
            transpose=True,
        )
        return f0, f1, (tp8, tq8)

    def warm(n):
        """Keep the PE p-state ramped with throwaway matmuls."""
        for _ in range(n):
            w = mp_ps.tile([128, 2, A], F32, tag="mp")
            nc.tensor.matmul(
                w[:, 0, 0:196], lhsT=dmy[:, 0:128], rhs=dmy[:, 0:196],
                start=True, stop=True,
            )

    # ---------------- setup ----------------
    setup_sb = ctx.enter_context(tc.tile_pool(name="setup_sb", bufs=1))
    h_sb = setup_sb.tile([BL, R], F32, tag="h_sb")

    # SP queue order = DMA priority order.
    nc.sync.dma_start(out=h_sb, in_=h_d)
    nc.sync.dma_start(out=bfh, in_=bfh_d)
    nc.sync.dma_start(out=wa_sb, in_=wa_d)
    nc.sync.dma_start(out=wf8, in_=wf_d)                  # 512KB, one shot
    nc.sync.dma_start(out=wh_bf[:, 0], in_=wh_d[:, 0])
    fb_tiles = {0: load_pair(0)}                          # SWDGE queue
    T_tiles = {0: xpose_pair(fb_tiles[0])}
    nc.sync.dma_start(out=wh_bf[:, 1], in_=wh_d[:, 1])
    nc.sync.dma_start(out=wh_bf[:, 2], in_=wh_d[:, 2])
    nc.sync.dma_start(out=wh_bf[:, 3], in_=wh_d[:, 3])
    fb_tiles[1] = load_pair(1)
    T_tiles[1] = xpose_pair(fb_tiles[1])
    fb_tiles[2] = load_pair(2)

    # DVE setup: warm rhs, reciprocal columns, res bank zeroing.
    nc.vector.memset(dmy, 0.0)
    nc.vector.memset(ones_row, 1.0)
    for t in rcol:
        nc.vector.memset(t, 1.0)
    for t in res_tiles:
        nc.vector.memset(t, 0.0)

    # PE: warm, then h transposes, then beta hc0; beta hc1-3 happen as
    # hooks inside pair 0's mm1 (as the Wh chunks land).
    warm(N_WARM0)

    hT_pt = sc_ps.tile([128, 512], F32, tag="sc")
    hT_ps = hT_pt[:, 0 : RC * BL]
    for rc in range(RC):
        nc.tensor.transpose(
            hT_ps[:, ts(rc, BL)], h_sb[:, ts(rc, 128)], ident[0:BL, 0:BL]
        )
    nc.vector.tensor_copy(out=hT, in_=hT_ps.rearrange("p (rc b) -> p rc b", rc=RC))

    bps4 = sc_ps.tile([128, HC, BL], F32, tag="sc")

    def beta_chunk(hc):
        for rc in range(RC):
            nc.tensor.matmul(
                bps4[:, hc, :],
                lhsT=wh_bf[:, hc, rc, :],
                rhs=hT[:, rc, :],
                start=(rc == 0),
                stop=False,
                skip_group_check=True,
            )
        nc.tensor.matmul(
            bps4[:, hc, :],
            lhsT=bfh[0:1, ts(hc, 128)],
            rhs=ones_row,
            start=False,
            stop=True,
            skip_group_check=True,
        )
        # ACT, not DVE: a DVE copy here waits on the Wh DMA and would
        # head-block the fp8 feats casts queued behind it
        nc.scalar.copy(out=betaT[:, hc], in_=bps4[:, hc])

    beta_chunk(0)
    warm(N_WARM1)

    # ---------------- steady state ----------------
    group_state = {}

    def tail_pieces(q, fb, dt_t):
        """Closures for pair q's softmax/mm2 tail, interleaved into pair
        q+1's mm1 chunks. No DVE on the critical path."""
        f0, f1 = fb
        g = q // 2
        if q % 2 == 0:
            gS_t = gsp.tile([1, 8], F32, tag="gS", name="gS")
            group_state[g] = {
                "res": (res_tiles[(g % 2) * 2], res_tiles[(g % 2) * 2 + 1]),
                "gS": gS_t,
            }
        gs = group_state[g]
        res_lo, res_hi = gs["res"]
        gS = gs["gS"]
        scb = sc_ps.tile([128, 512], F32, tag="sc")
        st = {}

        def p_scores():
            sc = scb[0:1, 0 : 2 * A]
            for hc in range(HC):
                nc.tensor.matmul(
                    sc,
                    lhsT=wa_sb[:, hc : hc + 1],
                    rhs=dt_t[:, hc, :, :],
                    start=(hc == 0),
                    stop=(hc == HC - 1),
                )
            er = erow.tile([1, 2 * A], F32, tag="er")
            for s in range(2):
                nc.scalar.activation(
                    out=er[0:1, ts(s, A)], in_=sc[0:1, ds(s * A, A)], func=EXP,
                    accum_out=gS[0:1, 2 * (q % 2) + s : 2 * (q % 2) + s + 1],
                )
            st["er"] = er

        def p_expt():
            # exp row -> psum columns -> one ACT copy to bf16 columns
            er = st["er"]
            et = scb[:, 400:404]
            for s in range(2):
                nc.tensor.transpose(
                    et[:, 2 * s : 2 * s + 1], er[0:1, ds(s * A, A0)], ident[0:1, 0:1]
                )
                nc.tensor.transpose(
                    et[0:A1, 2 * s + 1 : 2 * s + 2],
                    er[0:1, ds(s * A + A1_OFF, A1)],
                    ident[0:1, 0:1],
                )
            ec = ecp.tile([128, 4], BF16, tag="ec")
            nc.scalar.copy(out=ec[0:A1, :], in_=et[0:A1, :])
            nc.scalar.copy(out=ec[A1:128, 0:3:2], in_=et[A1:128, 0:3:2])
            st["ec"] = ec  # col 2s: e at a=0..128; col 2s+1: e at a=100..196
            if q % 2 == 1:
                # group complete: reciprocals of the 4 sums (DVE) and the
                # scatter to partitions {0,32,64,96}; both early so drain
                # never waits on them
                nc.vector.reciprocal(out=gS[0:1, 4:8], in_=gS[0:1, 0:4])
                # ACT's hwdge queue: never waits behind SP's big transposes
                nc.scalar.dma_start(out=rcol[g % 2][rows4, 0:1], in_=gS[0:1, 4:8])

        def p_mm2(s):
            b = 2 * q + s
            jb = b % 4
            ec = st["ec"]
            for res_t, cols in ((res_lo, ds(0, 512)), (res_hi, ds(512, 512))):
                nc.tensor.matmul(
                    res_t[ds(32 * jb, 1), :],
                    lhsT=ec[0:A1_OFF, 2 * s : 2 * s + 1],
                    rhs=f0[0:A1_OFF, s, cols],
                    start=True,
                    stop=False,
                    tile_position=(0, 32 * jb),
                )
                nc.tensor.matmul(
                    res_t[ds(32 * jb, 1), :],
                    lhsT=ec[0:A1, 2 * s + 1 : 2 * s + 2],
                    rhs=f1[:, s, cols],
                    start=False,
                    stop=True,
                    tile_position=(0, 32 * jb),
                )

        return [p_scores, p_expt, lambda: p_mm2(0), lambda: p_mm2(1)]

    def drain(g):
        """Scale by 1/sum on ACT and store. Runs during pair 2g+3 (or
        epilogue); by then every dependency (res, the reciprocal scatter)
        is a full pair old, so it can never stall ACT's tanh/exp queue."""
        gs = group_state.pop(g)
        res_lo, res_hi = gs["res"]
        # de-prioritize so the scheduler never sandwiches these between
        # latency-critical tanh/exp pairs on ACT
        with tc.high_priority(offset=-(10**6)):
            for res_t, half in ((res_lo, 0), (res_hi, 1)):
                stt = stp.tile([128, 512], F32, tag="st")
                nc.scalar.activation(
                    out=stt, in_=res_t, func=COPY, bias=0.0, scale=rcol[g % 2][:, 0:1]
                )
                # ACT's hwdge queue: keeps SP free for the transposes
                nc.scalar.dma_start(
                    out=out_d[ts(g, 4), ts(half, 512)], in_=stt[rows4, :]
                )

    def mm1_tanh(p, T, pieces, hooks):
        pair_b0 = 2 * p
        tp8, tq8 = T
        dt_t = dtp.tile([128, HC, 2, A], BF16, tag="dt_t")
        for hc in range(HC):
            mp = mp_ps.tile([128, 2, A], F32, tag="mp")
            # fp8 DoubleRow: each matmul contracts 256 r values (128
            # partitions x 2 byte-packed k-tiles). One PSUM-arming start
            # per bank, then pure accumulation.
            for c in range(4):
                for s in range(2):
                    t = 4 * s + c
                    rp = tp8[:, t].bitcast(FP8).rearrange("p (j k) -> p k j", k=2)
                    rq = tq8[:, t].bitcast(FP8).rearrange("p (j k) -> p k j", k=2)
                    nc.tensor.matmul(
                        mp[:, s, 0:A0],
                        lhsT=wf8[:, hc, c],
                        rhs=rp,
                        perf_mode=DROW,
                        start=(c == 0 and s == 0),
                        stop=False,
                        skip_group_check=True,
                    )
                    nc.tensor.matmul(
                        mp[:, s, A0:A],
                        lhsT=wf8[:, hc, c],
                        rhs=rq[:, :, A0 - A1_OFF : A1],
                        perf_mode=DROW,
                        start=False,
                        stop=(c == 3 and s == 1),
                        skip_group_check=True,
                    )
            for s in range(2):
                nc.scalar.activation(
                    out=dt_t[:, hc, s, :],
                    in_=mp[:, s, :],
                    func=TANH,
                    bias=betaT[:, hc, pair_b0 + s : pair_b0 + s + 1],
                    scale=1.0 / WF_SCALE,
                )
            if hc < len(pieces):
                pieces[hc]()
            for fn in hooks.get(hc, ()):
                fn()
        for piece in pieces[HC:]:
            piece()
        return dt_t

    # tail of pair q is interleaved into pair q+LAG's mm1 chunks: with the
    # fp8 mm1 a single pair no longer hides the ACT exp/ec latency chain
    pending = {}
    for p in range(NPAIRS):
        floor = tc.tile_wait_until(
            (T0_NS + p * PERIOD_NS) / 1e6, enable=PERIOD_NS > 0
        )
        floor.__enter__()
        f0_, f1_, T = T_tiles.pop(p)
        q_tail = p - LAG
        pieces = tail_pieces(*pending.pop(q_tail)) if q_tail in pending else []
        if p == NPAIRS - 1 and LAG > 1 and (p - 1) in pending:
            # last pair: also fold the LAG-1 tail in so the epilogue only
            # has one serial tail chain left
            pieces = pieces + tail_pieces(*pending.pop(p - 1))
        hooks = {}
        if p == 0:
            # finish beta as the Wh chunks land
            for hc in range(1, HC):
                hooks.setdefault(hc - 1, []).append(lambda c=hc: beta_chunk(c))
        # prefetch: loads 3 ahead, transposes 2 ahead
        if p + L_AHEAD < NPAIRS and p + L_AHEAD not in fb_tiles:
            hooks.setdefault(L_HOOK, []).append(
                lambda q=p + L_AHEAD: fb_tiles.__setitem__(q, load_pair(q))
            )
        q_t = p + T_AHEAD
        if q_t < NPAIRS and q_t not in T_tiles and q_t in fb_tiles:
            hooks.setdefault(T_HOOK, []).append(
                lambda q=q_t: T_tiles.__setitem__(q, xpose_pair(fb_tiles.pop(q)))
            )
        # drains: group g's mm2 completes in tail(2g+1) during pair 2g+1+LAG
        g_d = (p - 2 - LAG) // 2
        if p >= 2 + LAG and (p - LAG) % 2 == 1 and g_d in group_state:
            hooks.setdefault(3, []).append(lambda g=g_d: drain(g))
        dt_t = mm1_tanh(p, T, pieces, hooks)
        warm(WARM_AFTER.get(p, 0))
        pending[p] = (p, (f0_, f1_), dt_t)
        floor.__exit__(None, None, None)

    for q in sorted(pending):
        for piece in tail_pieces(*pending.pop(q)):
            piece()
    for g in sorted(group_state):
        drain(g)
    ctx.close()


_CACHE = {}


def _build():
    if "nc" in _CACHE:
        return _CACHE["nc"]
    nc = bacc.Bacc(
        "TRN2",
        target_bir_lowering=False,
        debug=False,
        enable_asserts=False,
        num_devices=N_CORES,
    )
    with tile.TileContext(nc) as tc:
        _emit(tc)
    nc.compile()
    _CACHE["nc"] = nc
    return nc


def _pack_weights(Wf, bf, Wh, bh, wa):
    """Host-side packing of the small replicated weights into the SBUF
    layouts the kernel streams in directly. Wf is quantized to fp8 e4m3 at
    x32 (values ~U(-1,1)) in the DoubleRow shuffle: [p, hc, c, kt, m] =
    32*Wf[256c + 2p + kt, 128hc + m]."""
    wf_p = np.ascontiguousarray(
        (Wf * WF_SCALE).reshape(4, 128, 2, HC, 128).transpose(1, 3, 0, 2, 4)
    ).astype(ml_dtypes.float8_e4m3)
    wh_p = np.ascontiguousarray(
        Wh.reshape(RC, 128, HC, 128).transpose(1, 2, 0, 3)
    ).astype(ml_dtypes.bfloat16)
    bfh_p = (bf + bh).reshape(1, H).astype(ml_dtypes.bfloat16)
    wa_p = np.ascontiguousarray(wa.reshape(HC, 128).T).astype(ml_dtypes.bfloat16)
    return wf_p, wh_p, bfh_p, wa_p


def kernel(h, feats, Wf, bf, Wh, bh, wa, ba=None, **_unused):
    h = np.ascontiguousarray(np.asarray(h, dtype=np.float32))
    feats = np.ascontiguousarray(np.asarray(feats, dtype=np.float32))
    Wf = np.ascontiguousarray(np.asarray(Wf, dtype=np.float32))
    bf = np.ascontiguousarray(np.asarray(bf, dtype=np.float32))
    Wh = np.ascontiguousarray(np.asarray(Wh, dtype=np.float32))
    bh = np.ascontiguousarray(np.asarray(bh, dtype=np.float32))
    wa = np.ascontiguousarray(np.asarray(wa, dtype=np.float32))

    nc = _build()
    wf_p, wh_p, bfh_p, wa_p = _pack_weights(Wf, bf, Wh, bh, wa)
    in_maps = []
    for i in range(N_CORES):
        sl = slice(i * BL, (i + 1) * BL)
        in_maps.append(
            {
                "h_in": np.ascontiguousarray(h[sl]),
                "feats_in": np.ascontiguousarray(feats[sl]),
                "wf_in": wf_p,
                "wh_in": wh_p,
                "bfh_in": bfh_p,
                "wa_in": wa_p,
            }
        )
    res = run_bass_kernel_spmd(nc, in_maps, core_ids=list(range(N_CORES)))
    out = np.concatenate([res.results[i]["out"] for i in range(N_CORES)], axis=0)
    return out.astype(np.float32)


if __name__ == "__main__":
    rng = np.random.default_rng(0)
    s_f = 1.0 / np.sqrt(R)
    s_a = 1.0 / np.sqrt(H)
    inputs = {
        "h": rng.standard_normal((B, R), dtype=np.float32),
        "feats": rng.standard_normal((B, A, R), dtype=np.float32),
        "Wf": rng.uniform(-s_f, s_f, (R, H)).astype(np.float32),
        "bf": rng.uniform(-s_f, s_f, (H,)).astype(np.float32),
        "Wh": rng.uniform(-s_f, s_f, (R, H)).astype(np.float32),
        "bh": rng.uniform(-s_f, s_f, (H,)).astype(np.float32),
        "wa": rng.uniform(-s_a, s_a, (H,)).astype(np.float32),
        "ba": np.float32(0.1),
    }
    out = kernel(**inputs)
    print(out.shape, out.dtype, np.abs(out).mean())
